# revision 7
# baseline (speedup 1.0000x reference)
"""Pairwise Euclidean distance kernel for Trainium2 (8 NeuronCores, SPMD).

Problem: mapping [8192, 256] f32 -> out [8192, 8192] f32 where
out[i, j] = ||mapping[i] - mapping[j]||_2, via the GEMM identity
d2 = ||x_i||^2 + ||x_j||^2 - 2 <x_i, x_j>.

V3 = V2 (symmetric/triangle, f16 output) + sequencer/overhead engineering.
V2's trace showed PE.SEQ 100% busy (Matmult 42us exec + Ldweights 24us +
sems 12us), SP.SEQ 73us issuing 87 DMAs, HWDGE 54us of per-DMA fixed cost,
ACT 51us. V3:
  - [128, 1536] PSUM chunks (3 banks x 2 bufs + a separate 2-bank ring for
    the sq transposes): 24 uniform chunk-rows, no ragged 512 tails; 24
    epilogue ACT ops and 24 output DMAs instead of 40 each.
  - matmuls grouped by stationary operand (lhs0 x3 subs, lhs1 x3, ones x3)
    so post-schedule legalization skips repeated Ldweights.
  - sq hi/lo flattened with ONE PE transpose per group ([128, 8] ->
    [8, 128], hi in cols 0:4, lo in 4:8) and ONE strided DMA per group.
  - input DMAs merged: mt in 1024/2048-col slices (8), nat in 1024-row
    pairs (5).
  - output stores issued from the otherwise-idle Pool engine (SWDGE path),
    off the SP sequencer and the shared HWDGE unit.

Scheme recap: core c is rotated so its own 1024 rows sit first; for each
512-row half h it computes columns [h*512, h*512+4608) of its rotated tile
(unit a covers column units a..a+8 mod 16; every pair is covered directly
or by the transpose of its mirror; the host mirrors the remaining 112
blocks). Output f16 (rel err ~5e-4 vs the 2e-2 gate), widened on the host.

Hardware pitfalls (this container's TRN2 + neuronxcc build):
  - InstTensorTensorReduce (fused DVE square+reduce) and ACT Square with
    accum_out both crash the device (NRT_EXEC_UNIT_UNRECOVERABLE); use
    plain Square + separate reduce_sum instead.
  - ACT Sqrt on negative inputs yields NaN (CoreSim asserts); clamp first.
"""

import sys

try:
    import concourse.bass as _probe  # noqa: F401
except ImportError:
    sys.path.insert(0, "/opt/trn_rl_repo")

import numpy as np

import concourse.bacc as bacc
import concourse.mybir as mybir
from concourse import tile
from concourse.bass_utils import run_bass_kernel_spmd

N = 8192          # number of points
D = 256           # feature dim
NCORES = 8
RPC = N // NCORES  # 1024 rows per core
U = 512            # unit = 512 rows/cols
SPAN_U = 9         # column units covered per 512-row half
NCOL = 10 * U      # columns of mt/nat each core holds (5120)
NG = 10            # sq groups of 512 rows each
NPAIR = 5          # nat load pairs (1024 rows)
NUNITS = N // U    # 16 global units
CW = 3 * U         # chunk width 1536

F16 = mybir.dt.float16
F32 = mybir.dt.float32

# entry schedule: (half, (units...)) — 1536-wide, ordered so entry i's
# chains/loads are prefetched during earlier entries
ENTRIES = [
    (0, (0, 1, 2)), (1, (1, 2, 3)),
    (0, (3, 4, 5)), (1, (4, 5, 6)),
    (0, (6, 7, 8)), (1, (7, 8, 9)),
]
# sq chains to emit inside each entry (groups 0-2 run before entry 0)
CHAINS = {0: (3, 4), 1: (5, 6), 2: (7,), 3: (8,), 4: (9,)}


def _build_nc(repeats=1, loop_n=None, stage_bufs=4, work_bufs=2):
    nc = bacc.Bacc(None, target_bir_lowering=False)
    mt_d = nc.dram_tensor("mt", [D, NCOL], F16, kind="ExternalInput")
    nat_d = nc.dram_tensor("nat", [NCOL, D], F16, kind="ExternalInput")
    eye_d = nc.dram_tensor("eye", [128, 128], F32, kind="ExternalInput")
    out_d = nc.dram_tensor("out", [RPC, NCOL], F16, kind="ExternalOutput")

    with tile.TileContext(nc) as tc:
        with (
            tc.tile_pool(name="big", bufs=1) as big,
            tc.tile_pool(name="work", bufs=work_bufs) as work,
            tc.tile_pool(name="stage", bufs=stage_bufs) as stage_pool,
            tc.tile_pool(name="ps", bufs=2, space="PSUM") as psum,
        ):
            if loop_n is not None:
                with tc.For_i(0, loop_n, 1):
                    _emit_body(nc, tc, big, work, stage_pool, psum,
                               mt_d, nat_d, eye_d, out_d)
            else:
                for _rep in range(repeats):
                    _emit_body(nc, tc, big, work, stage_pool, psum,
                               mt_d, nat_d, eye_d, out_d)

    nc.compile()
    return nc


def _emit_body(nc, tc, big, work, stage_pool, psum, mt_d, nat_d, eye_d, out_d):
    mt0 = big.tile([128, NCOL], F16, tag="mt0")
    mt1 = big.tile([128, NCOL], F16, tag="mt1")
    eye = big.tile([128, 128], F32, tag="eye")
    ones2 = big.tile([2, 128], F16, tag="ones2")
    # per-group sq tensors: a shared tile would create false WAR/RAW
    # couplings, serializing the pipeline
    sqp = []
    sqf = []
    for _g in range(NG):
        sqp_t = big.tile([128, 4], F32, tag=f"sqp{_g}")
        sqp.append(sqp_t)
        sqf_t = big.tile([2, U], F16, tag=f"sqf{_g}")
        sqf.append(sqf_t)
    half_own = big.tile([128, 8], F32, tag="half_own")

    natp = nat_d.rearrange("(q t p) d -> q p t d", q=NPAIR, p=128)
    gtp = {}
    for _q in range(NPAIR):
        gt_slot = big.tile([128, 8, 256], F16, tag=f"natp{_q}")
        gtp[_q] = gt_slot

    # initial loads: nat pairs 0-1 (sq groups 0..3) + mt units 0..3; unit 0
    # split out so the first k-matmuls (stationary AND moving both live in
    # unit 0) start after a 1KB/partition transfer instead of 4KB
    nc.sync.dma_start(gtp[0][:], natp[0])
    nc.sync.dma_start(mt0[:, 0:U], mt_d[0:128, 0:U])
    nc.sync.dma_start(mt1[:, 0:U], mt_d[128:256, 0:U])
    nc.sync.dma_start(mt0[:, U:2048], mt_d[0:128, U:2048])
    nc.sync.dma_start(mt1[:, U:2048], mt_d[128:256, U:2048])
    nc.sync.dma_start(gtp[1][:], natp[1])
    nc.sync.dma_start(eye[:], eye_d[:])

    def emit_loads(ei):
        if ei == 0:
            nc.sync.dma_start(gtp[2][:], natp[2])
            nc.sync.dma_start(mt0[:, 2048:3072], mt_d[0:128, 2048:3072])
            nc.sync.dma_start(mt1[:, 2048:3072], mt_d[128:256, 2048:3072])
        elif ei == 1:
            nc.sync.dma_start(gtp[3][:], natp[3])
            nc.sync.dma_start(mt0[:, 3072:4096], mt_d[0:128, 3072:4096])
            nc.sync.dma_start(mt1[:, 3072:4096], mt_d[128:256, 3072:4096])
        elif ei == 2:
            nc.sync.dma_start(gtp[4][:], natp[4])
            nc.sync.dma_start(mt0[:, 4096:NCOL], mt_d[0:128, 4096:NCOL])
            nc.sync.dma_start(mt1[:, 4096:NCOL], mt_d[128:256, 4096:NCOL])

    def emit_sq_reduce(g):
        gt = gtp[g // 2][:, (g % 2) * 4:(g % 2) * 4 + 4, :]
        # square on ACT (plain Square — fused/accum variants crash this HW),
        # reduce on DVE
        msq = work.tile([128, 4, 256], F32, tag="msq")
        nc.scalar.activation(msq[:], gt,
                             mybir.ActivationFunctionType.Square)
        nc.vector.reduce_sum(sqp[g][:, 0:4].unsqueeze(2), msq[:],
                             axis=mybir.AxisListType.X)
        # -0.5*sq split hi/lo (exact to ~2^-22): hi source in cols 0:4,
        # f16 residual in cols 4:8 of one tile so a single PE transpose
        # flattens both rows
        mhl = work.tile([128, 8], F32, tag=f"mhl{g}")
        nc.vector.tensor_scalar_mul(mhl[:, 0:4], sqp[g][:, 0:4], -0.5)
        hi16 = work.tile([128, 4], F16, tag="hi16")
        nc.vector.tensor_copy(hi16[:], mhl[:, 0:4])
        hi32 = work.tile([128, 4], F32, tag="hi32")
        nc.vector.tensor_copy(hi32[:], hi16[:])
        nc.vector.tensor_sub(mhl[:, 4:8], mhl[:, 0:4], hi32[:])
        if g < 2:
            # own-rows 0.5*sq_i for the diagonal clamp
            nc.vector.tensor_scalar_mul(half_own[:, g * 4:(g + 1) * 4],
                                        sqp[g][:, 0:4], 0.5)
        return mhl

    def emit_sq_flatten(g, mhl):
        # one PE transpose [128, 8] -> [8, 128] (partition 4r+t holds row
        # r's tile-t slab), one f16 copy, one flatten DMA
        pt = psum.tile([8, 128], F32, tag="pst")
        nc.tensor.transpose(pt[:], mhl[:], eye[:])
        st = work.tile([8, 128], F16, tag="sqT")
        nc.vector.tensor_copy(st[:], pt[:])
        nc.sync.dma_start(
            sqf[g].rearrange("r (t i) -> r t i", t=4),
            st[:],
        )

    def emit_chain(g):
        emit_sq_flatten(g, emit_sq_reduce(g))

    def emit_kmms(ps, r, units):
        # grouped by stationary so legalization drops repeated Ldweights
        lhs0 = mt0[:, r * 128:(r + 1) * 128]
        lhs1 = mt1[:, r * 128:(r + 1) * 128]
        for s, u in enumerate(units):
            j = u * U
            nc.tensor.matmul(ps[:, s * U:(s + 1) * U], lhs0,
                             mt0[:, j:j + U], start=True, stop=False)
        for s, u in enumerate(units):
            j = u * U
            nc.tensor.matmul(ps[:, s * U:(s + 1) * U], lhs1,
                             mt1[:, j:j + U], start=False, stop=False)

    def emit_rank1(ps, units):
        for s, u in enumerate(units):
            nc.tensor.matmul(ps[:, s * U:(s + 1) * U], ones2[:],
                             sqf[u][:, :], start=False, stop=True)

    def emit_tail(ps, r, units, on_pool):
        out_t = stage_pool.tile([128, CW], F16, tag="stage")
        bias = sqp[r // 4][:, r % 4:r % 4 + 1]
        if r // 4 in units:
            # diagonal block: clamp psum <= 0.5*sq_i so ACT Sqrt input
            # -2*psum + sq_i stays >= 0 under fp rounding
            s = units.index(r // 4)
            off = s * U + (r % 4) * 128
            dg = ps[:, off:off + 128]
            nc.vector.tensor_scalar_min(dg, dg, half_own[:, r:r + 1])
        nc.scalar.activation(
            out_t[:], ps[:],
            mybir.ActivationFunctionType.Sqrt,
            bias=bias, scale=-2.0,
        )
        # stores alternate Pool (SWDGE, ~2.8us desc-gen but otherwise idle
        # engine) and SP (HWDGE): either queue alone would throttle the
        # ~2.7us/row pipeline; the last row of each entry goes to SP so the
        # final store is not behind a slow SWDGE gen
        eng = nc.gpsimd if on_pool else nc.sync
        eng.dma_start(
            out_d[r * 128:(r + 1) * 128,
                  units[0] * U:units[0] * U + CW],
            out_t[:],
        )

    nc.vector.memset(ones2[:], 1.0)
    # groups 0-2 reduces first at high priority (DVE/ACT only); their PE
    # transposes are deferred until after the first k-matmul runway so the
    # in-order PE is never parked behind the DVE chain
    with tc.high_priority():
        m0 = emit_sq_reduce(0)
        m1 = emit_sq_reduce(1)
        m2 = emit_sq_reduce(2)

    for ei, (h, units) in enumerate(ENTRIES):
        rows = [4 * h + k for k in range(4)]
        chain = CHAINS.get(ei, ())
        if ei == 0:
            ps0 = psum.tile([128, CW], F32, tag="ps")
            emit_kmms(ps0, rows[0], units)
            emit_loads(0)
            ps1 = psum.tile([128, CW], F32, tag="ps")
            emit_kmms(ps1, rows[1], units)
            emit_sq_flatten(0, m0)
            emit_sq_flatten(1, m1)
            emit_sq_flatten(2, m2)
            emit_rank1(ps0, units)
            emit_tail(ps0, rows[0], units, on_pool=True)
            emit_rank1(ps1, units)
            emit_tail(ps1, rows[1], units, on_pool=False)
            for idx, r in enumerate(rows[2:]):
                if idx < len(chain):
                    emit_chain(chain[idx])
                ps = psum.tile([128, CW], F32, tag="ps")
                emit_kmms(ps, r, units)
                emit_rank1(ps, units)
                emit_tail(ps, r, units, on_pool=(idx == 0))
            continue
        for idx, r in enumerate(rows):
            if idx == 0:
                emit_loads(ei)
            if idx in (1, 3) and len(chain) > idx // 2:
                emit_chain(chain[idx // 2])
            ps = psum.tile([128, CW], F32, tag="ps")
            emit_kmms(ps, r, units)
            emit_rank1(ps, units)
            emit_tail(ps, r, units, on_pool=(idx in (0, 2)))


_NC_CACHE = None


def _get_nc():
    global _NC_CACHE
    if _NC_CACHE is None:
        _NC_CACHE = _build_nc()
    return _NC_CACHE


def prep_inputs(mapping: np.ndarray) -> list:
    xh = mapping.astype(np.float16)
    eye = np.eye(128, dtype=np.float32)
    in_maps = []
    for c in range(NCORES):
        rot = np.roll(xh, -c * RPC, axis=0)
        natc = np.ascontiguousarray(rot[0:NCOL])
        mtc = np.ascontiguousarray(natc.T)
        in_maps.append({"mt": mtc, "nat": natc, "eye": eye})
    return in_maps


def kernel(mapping: np.ndarray, **_kwargs) -> np.ndarray:
    mapping = np.asarray(mapping, dtype=np.float32)
    assert mapping.shape == (N, D)
    in_maps = prep_inputs(mapping)

    nc = _get_nc()
    res = run_bass_kernel_spmd(nc, in_maps, core_ids=list(range(NCORES)))

    out = np.empty((N, N), dtype=np.float32)
    covered = np.zeros((NUNITS, NUNITS), dtype=bool)
    span = SPAN_U * U
    for c in range(NCORES):
        oc = res.results[c]["out"]  # [1024, 5120] f16
        for h in (0, 1):
            au = c * 2 + h
            block = oc[h * U:(h + 1) * U, h * U:h * U + span].astype(np.float32)
            gr0 = c * RPC + h * U
            gc0 = (c * RPC + h * U) % N
            first = min(span, N - gc0)
            out[gr0:gr0 + U, gc0:gc0 + first] = block[:, :first]
            if first < span:
                out[gr0:gr0 + U, 0:span - first] = block[:, first:]
            for cu in range(SPAN_U):
                covered[au, (au + cu) % NUNITS] = True
    for a in range(NUNITS):
        for b in range(NUNITS):
            if not covered[a, b]:
                out[a * U:(a + 1) * U, b * U:(b + 1) * U] = \
                    out[b * U:(b + 1) * U, a * U:(a + 1) * U].T
    return out


if __name__ == "__main__":
    rng = np.random.default_rng(0)
    x = rng.standard_normal((N, D)).astype(np.float32)
    o = kernel(mapping=x)
    sq = (x * x).sum(1)
    ref = np.sqrt(np.maximum(sq[:, None] + sq[None, :] - 2 * x @ x.T, 0))
    d = np.abs(o - ref)
    print("out", o.shape, o.dtype, "absmax diff", d.max(),
          "diag", np.abs(np.diag(o)).max())


# revision 16
# speedup vs baseline: 1.0724x; 1.0724x over previous
"""Pairwise Euclidean distance kernel for Trainium2 (8 NeuronCores, SPMD).

Problem: mapping [8192, 256] f32 -> out [8192, 8192] f32 where
out[i, j] = ||mapping[i] - mapping[j]||_2, via the GEMM identity
d2 = ||x_i||^2 + ||x_j||^2 - 2 <x_i, x_j>.

V3 = V2 (symmetric/triangle, f16 output) + sequencer/overhead engineering.
V2's trace showed PE.SEQ 100% busy (Matmult 42us exec + Ldweights 24us +
sems 12us), SP.SEQ 73us issuing 87 DMAs, HWDGE 54us of per-DMA fixed cost,
ACT 51us. V3:
  - [128, 1536] PSUM chunks (3 banks x 2 bufs + a separate 2-bank ring for
    the sq transposes): 24 uniform chunk-rows, no ragged 512 tails; 24
    epilogue ACT ops and 24 output DMAs instead of 40 each.
  - sq hi/lo flattened with ONE PE transpose per group ([128, 8] ->
    [8, 128], hi in cols 0:4, lo in 4:8) and ONE strided DMA per group.
  - input DMAs merged: mt in 1024/2048-col slices (8), nat in 1024-row
    pairs (5).
  - output stores issued from the otherwise-idle Pool engine (SWDGE path),
    off the SP sequencer (where a data-dependent store would block the
    prefetch loads queued behind it) and the shared HWDGE unit.
  - final row drains as 3 interleaved rank1+sqrt+store 512 slices, so the
    serial tail is one slice (ACT-issued store, empty queue behind it).

Scheme recap: core c is rotated so its own 1024 rows sit first; for each
512-row half h it computes columns [h*512, h*512+4608) of its rotated tile
(unit a covers column units a..a+8 mod 16; every pair is covered directly
or by the transpose of its mirror; the host mirrors the remaining 112
blocks). Output f16 (rel err ~5e-4 vs the 2e-2 gate), widened on the host.

Hardware pitfalls (this container's TRN2 + neuronxcc build):
  - InstTensorTensorReduce (fused DVE square+reduce) and ACT Square with
    accum_out both crash the device (NRT_EXEC_UNIT_UNRECOVERABLE); use
    plain Square + separate reduce_sum instead.
  - ACT Sqrt on negative inputs yields NaN (CoreSim asserts); clamp first.
"""

import sys

try:
    import concourse.bass as _probe  # noqa: F401
except ImportError:
    sys.path.insert(0, "/opt/trn_rl_repo")

import numpy as np

import concourse.bacc as bacc
import concourse.mybir as mybir
from concourse import tile
from concourse.bass_utils import run_bass_kernel_spmd

N = 8192          # number of points
D = 256           # feature dim
NCORES = 8
RPC = N // NCORES  # 1024 rows per core
U = 512            # unit = 512 rows/cols
SPAN_U = 9         # column units covered per 512-row half
NCOL = 10 * U      # columns of mt/nat each core holds (5120)
NG = 10            # sq groups of 512 rows each
NPAIR = 5          # nat load pairs (1024 rows)
NUNITS = N // U    # 16 global units
CW = 3 * U         # chunk width 1536

F16 = mybir.dt.float16
F32 = mybir.dt.float32

# entry schedule: (half, (units...)) — 1536-wide, ordered so entry i's
# chains/loads are prefetched during earlier entries
ENTRIES = [
    (0, (0, 1, 2)), (1, (1, 2, 3)),
    (0, (3, 4, 5)), (1, (4, 5, 6)),
    (0, (6, 7, 8)), (1, (7, 8, 9)),
]
# sq chains to emit inside each entry (groups 0-2 run before entry 0)
CHAINS = {0: (3, 4), 1: (5, 6), 2: (7,), 3: (8,), 4: (9,)}


def _build_nc(repeats=1, loop_n=None, stage_bufs=4, work_bufs=2):
    nc = bacc.Bacc(None, target_bir_lowering=False)
    mt_d = nc.dram_tensor("mt", [D, NCOL], F16, kind="ExternalInput")
    nat_d = nc.dram_tensor("nat", [NCOL, D], F16, kind="ExternalInput")
    eye_d = nc.dram_tensor("eye", [128, 128], F32, kind="ExternalInput")
    out_d = nc.dram_tensor("out", [RPC, NCOL], F16, kind="ExternalOutput")

    with tile.TileContext(nc) as tc:
        with (
            tc.tile_pool(name="big", bufs=1) as big,
            tc.tile_pool(name="work", bufs=work_bufs) as work,
            tc.tile_pool(name="stage", bufs=stage_bufs) as stage_pool,
            tc.tile_pool(name="ps", bufs=2, space="PSUM") as psum,
        ):
            if loop_n is not None:
                with tc.For_i(0, loop_n, 1):
                    _emit_body(nc, tc, big, work, stage_pool, psum,
                               mt_d, nat_d, eye_d, out_d)
            else:
                for _rep in range(repeats):
                    _emit_body(nc, tc, big, work, stage_pool, psum,
                               mt_d, nat_d, eye_d, out_d)

    nc.compile()
    return nc


def _emit_body(nc, tc, big, work, stage_pool, psum, mt_d, nat_d, eye_d, out_d):
    mt0 = big.tile([128, NCOL], F16, tag="mt0")
    mt1 = big.tile([128, NCOL], F16, tag="mt1")
    eye = big.tile([128, 128], F32, tag="eye")
    ones2 = big.tile([2, 128], F16, tag="ones2")
    # per-group sq tensors: a shared tile would create false WAR/RAW
    # couplings, serializing the pipeline
    sqp = []
    sqf = []
    for _g in range(NG):
        sqp_t = big.tile([128, 4], F32, tag=f"sqp{_g}")
        sqp.append(sqp_t)
        sqf_t = big.tile([2, U], F16, tag=f"sqf{_g}")
        sqf.append(sqf_t)
    half_own = big.tile([128, 8], F32, tag="half_own")

    natp = nat_d.rearrange("(q t p) d -> q p t d", q=NPAIR, p=128)
    gtp = {}
    for _q in range(NPAIR):
        gt_slot = big.tile([128, 8, 256], F16, tag=f"natp{_q}")
        gtp[_q] = gt_slot

    # initial loads: nat pairs 0-1 (sq groups 0..3) + mt units 0..3.
    # NOTE: reordering these (mt first) or splitting them finer lowers the
    # first-PE-op time from 7.2us to ~3.5us but LOSES overall: the idle is
    # then spread over several small mid-stream gaps, and every gap resets
    # the PE p-state ramp (3us of half-clock matmuls per gap). One
    # consolidated idle block at the start is the cheapest place to wait.
    nc.sync.dma_start(gtp[0][:], natp[0])
    nc.sync.dma_start(gtp[1][:], natp[1])
    nc.sync.dma_start(mt0[:, 0:2048], mt_d[0:128, 0:2048])
    nc.sync.dma_start(mt1[:, 0:2048], mt_d[128:256, 0:2048])
    nc.sync.dma_start(eye[:], eye_d[:])

    def emit_loads(ei):
        if ei == 0:
            nc.sync.dma_start(gtp[2][:], natp[2])
            nc.sync.dma_start(mt0[:, 2048:3072], mt_d[0:128, 2048:3072])
            nc.sync.dma_start(mt1[:, 2048:3072], mt_d[128:256, 2048:3072])
        elif ei == 1:
            nc.sync.dma_start(gtp[3][:], natp[3])
            nc.sync.dma_start(mt0[:, 3072:4096], mt_d[0:128, 3072:4096])
            nc.sync.dma_start(mt1[:, 3072:4096], mt_d[128:256, 3072:4096])
        elif ei == 2:
            nc.sync.dma_start(gtp[4][:], natp[4])
            nc.sync.dma_start(mt0[:, 4096:NCOL], mt_d[0:128, 4096:NCOL])
            nc.sync.dma_start(mt1[:, 4096:NCOL], mt_d[128:256, 4096:NCOL])

    def emit_sq_reduce(g):
        gt = gtp[g // 2][:, (g % 2) * 4:(g % 2) * 4 + 4, :]
        # square on ACT (plain Square — fused/accum variants crash this HW),
        # reduce on DVE
        msq = work.tile([128, 4, 256], F32, tag="msq")
        nc.scalar.activation(msq[:], gt,
                             mybir.ActivationFunctionType.Square)
        nc.vector.reduce_sum(sqp[g][:, 0:4].unsqueeze(2), msq[:],
                             axis=mybir.AxisListType.X)
        # -0.5*sq split hi/lo (exact to ~2^-22): hi source in cols 0:4,
        # f16 residual in cols 4:8 of one tile so a single PE transpose
        # flattens both rows
        mhl = work.tile([128, 8], F32, tag=f"mhl{g}")
        nc.vector.tensor_scalar_mul(mhl[:, 0:4], sqp[g][:, 0:4], -0.5)
        hi16 = work.tile([128, 4], F16, tag="hi16")
        nc.vector.tensor_copy(hi16[:], mhl[:, 0:4])
        hi32 = work.tile([128, 4], F32, tag="hi32")
        nc.vector.tensor_copy(hi32[:], hi16[:])
        nc.vector.tensor_sub(mhl[:, 4:8], mhl[:, 0:4], hi32[:])
        if g < 2:
            # own-rows 0.5*sq_i for the diagonal clamp
            nc.vector.tensor_scalar_mul(half_own[:, g * 4:(g + 1) * 4],
                                        sqp[g][:, 0:4], 0.5)
        return mhl

    def emit_sq_flatten(g, mhl):
        # one PE transpose [128, 8] -> [8, 128] (partition 4r+t holds row
        # r's tile-t slab), one f16 copy, one flatten DMA
        pt = psum.tile([8, 128], F32, tag="pst")
        nc.tensor.transpose(pt[:], mhl[:], eye[:])
        st = work.tile([8, 128], F16, tag="sqT")
        nc.vector.tensor_copy(st[:], pt[:])
        nc.sync.dma_start(
            sqf[g].rearrange("r (t i) -> r t i", t=4),
            st[:],
        )

    def emit_chain(g):
        emit_sq_flatten(g, emit_sq_reduce(g))

    def emit_kmms(ps, r, units):
        lhs0 = mt0[:, r * 128:(r + 1) * 128]
        lhs1 = mt1[:, r * 128:(r + 1) * 128]
        for s, u in enumerate(units):
            j = u * U
            nc.tensor.matmul(ps[:, s * U:(s + 1) * U], lhs0,
                             mt0[:, j:j + U], start=True, stop=False)
        for s, u in enumerate(units):
            j = u * U
            nc.tensor.matmul(ps[:, s * U:(s + 1) * U], lhs1,
                             mt1[:, j:j + U], start=False, stop=False)

    def emit_rank1(ps, units):
        for s, u in enumerate(units):
            nc.tensor.matmul(ps[:, s * U:(s + 1) * U], ones2[:],
                             sqf[u][:, :], start=False, stop=True)

    def emit_tail(ps, r, units):
        out_t = stage_pool.tile([128, CW], F16, tag="stage")
        bias = sqp[r // 4][:, r % 4:r % 4 + 1]
        if r // 4 in units:
            # diagonal block: clamp psum <= 0.5*sq_i so ACT Sqrt input
            # -2*psum + sq_i stays >= 0 under fp rounding
            s = units.index(r // 4)
            off = s * U + (r % 4) * 128
            dg = ps[:, off:off + 128]
            nc.vector.tensor_scalar_min(dg, dg, half_own[:, r:r + 1])
        nc.scalar.activation(
            out_t[:], ps[:],
            mybir.ActivationFunctionType.Sqrt,
            bias=bias, scale=-2.0,
        )
        # store from the Pool engine: SWDGE path, keeps the 24 output DMAs
        # off the SP sequencer and the shared HWDGE unit
        nc.gpsimd.dma_start(
            out_d[r * 128:(r + 1) * 128,
                  units[0] * U:units[0] * U + CW],
            out_t[:],
        )

    nc.vector.memset(ones2[:], 1.0)
    with tc.high_priority():
        emit_chain(0)
        emit_chain(1)

    for ei, (h, units) in enumerate(ENTRIES):
        rows = [4 * h + k for k in range(4)]
        chain = CHAINS.get(ei, ())
        if ei == 0:
            # runway: slot 0's k-matmuls stream while the group-2 chain
            # completes; rank-1s join once sqf lands
            ps0 = psum.tile([128, CW], F32, tag="ps")
            emit_kmms(ps0, rows[0], units)
            with tc.high_priority():
                emit_chain(2)
            emit_loads(0)
            ps1 = psum.tile([128, CW], F32, tag="ps")
            emit_kmms(ps1, rows[1], units)
            emit_rank1(ps0, units)
            emit_tail(ps0, rows[0], units)
            emit_rank1(ps1, units)
            emit_tail(ps1, rows[1], units)
            for idx, r in enumerate(rows[2:]):
                if idx < len(chain):
                    emit_chain(chain[idx])
                ps = psum.tile([128, CW], F32, tag="ps")
                emit_kmms(ps, r, units)
                emit_rank1(ps, units)
                emit_tail(ps, r, units)
            continue
        for idx, r in enumerate(rows):
            if idx == 0:
                emit_loads(ei)
            if idx in (1, 3) and len(chain) > idx // 2:
                emit_chain(chain[idx // 2])
            ps = psum.tile([128, CW], F32, tag="ps")
            emit_kmms(ps, r, units)
            if ei == len(ENTRIES) - 1 and idx == 3:
                # final row: interleave each 512 sub's rank-1 with its own
                # epilogue act + store so the drain tail is one slice, not
                # a whole 1536 chunk; the very last store is ACT-issued
                # (fast HWDGE gen, and the ACT queue behind it is empty)
                bias = sqp[r // 4][:, r % 4:r % 4 + 1]
                for s, u in enumerate(units):
                    nc.tensor.matmul(ps[:, s * U:(s + 1) * U], ones2[:],
                                     sqf[u][:, :], start=False, stop=True)
                    out_s = stage_pool.tile([128, U], F16, tag="stage_s")
                    nc.scalar.activation(
                        out_s[:], ps[:, s * U:(s + 1) * U],
                        mybir.ActivationFunctionType.Sqrt,
                        bias=bias, scale=-2.0,
                    )
                    eng = nc.scalar if s == len(units) - 1 else nc.gpsimd
                    eng.dma_start(
                        out_d[r * 128:(r + 1) * 128, u * U:(u + 1) * U],
                        out_s[:],
                    )
            else:
                emit_rank1(ps, units)
                emit_tail(ps, r, units)


_NC_CACHE = None


def _get_nc():
    global _NC_CACHE
    if _NC_CACHE is None:
        _NC_CACHE = _build_nc()
    return _NC_CACHE


def prep_inputs(mapping: np.ndarray) -> list:
    xh = mapping.astype(np.float16)
    eye = np.eye(128, dtype=np.float32)
    in_maps = []
    for c in range(NCORES):
        rot = np.roll(xh, -c * RPC, axis=0)
        natc = np.ascontiguousarray(rot[0:NCOL])
        mtc = np.ascontiguousarray(natc.T)
        in_maps.append({"mt": mtc, "nat": natc, "eye": eye})
    return in_maps


def kernel(mapping: np.ndarray, **_kwargs) -> np.ndarray:
    mapping = np.asarray(mapping, dtype=np.float32)
    assert mapping.shape == (N, D)
    in_maps = prep_inputs(mapping)

    nc = _get_nc()
    res = run_bass_kernel_spmd(nc, in_maps, core_ids=list(range(NCORES)))

    out = np.empty((N, N), dtype=np.float32)
    covered = np.zeros((NUNITS, NUNITS), dtype=bool)
    span = SPAN_U * U
    for c in range(NCORES):
        oc = res.results[c]["out"]  # [1024, 5120] f16
        for h in (0, 1):
            au = c * 2 + h
            block = oc[h * U:(h + 1) * U, h * U:h * U + span].astype(np.float32)
            gr0 = c * RPC + h * U
            gc0 = (c * RPC + h * U) % N
            first = min(span, N - gc0)
            out[gr0:gr0 + U, gc0:gc0 + first] = block[:, :first]
            if first < span:
                out[gr0:gr0 + U, 0:span - first] = block[:, first:]
            for cu in range(SPAN_U):
                covered[au, (au + cu) % NUNITS] = True
    for a in range(NUNITS):
        for b in range(NUNITS):
            if not covered[a, b]:
                out[a * U:(a + 1) * U, b * U:(b + 1) * U] = \
                    out[b * U:(b + 1) * U, a * U:(a + 1) * U].T
    return out


if __name__ == "__main__":
    rng = np.random.default_rng(0)
    x = rng.standard_normal((N, D)).astype(np.float32)
    o = kernel(mapping=x)
    sq = (x * x).sum(1)
    ref = np.sqrt(np.maximum(sq[:, None] + sq[None, :] - 2 * x @ x.T, 0))
    d = np.abs(o - ref)
    print("out", o.shape, o.dtype, "absmax diff", d.max(),
          "diag", np.abs(np.diag(o)).max())


# revision 18
# speedup vs baseline: 1.0782x; 1.0054x over previous
"""Pairwise Euclidean distance kernel for Trainium2 (8 NeuronCores, SPMD).

Problem: mapping [8192, 256] f32 -> out [8192, 8192] f32 where
out[i, j] = ||mapping[i] - mapping[j]||_2, via the GEMM identity
d2 = ||x_i||^2 + ||x_j||^2 - 2 <x_i, x_j>.

V3 = V2 (symmetric/triangle, f16 output) + sequencer/overhead engineering.
V2's trace showed PE.SEQ 100% busy (Matmult 42us exec + Ldweights 24us +
sems 12us), SP.SEQ 73us issuing 87 DMAs, HWDGE 54us of per-DMA fixed cost,
ACT 51us. V3:
  - [128, 1536] PSUM chunks (3 banks x 2 bufs + a separate 2-bank ring for
    the sq transposes): 24 uniform chunk-rows, no ragged 512 tails; 24
    epilogue ACT ops and 24 output DMAs instead of 40 each.
  - sq hi/lo flattened with ONE PE transpose per group ([128, 8] ->
    [8, 128], hi in cols 0:4, lo in 4:8) and ONE strided DMA per group.
  - input DMAs merged: mt in 1024/2048-col slices (8), nat in 1024-row
    pairs (5).
  - output stores issued from the otherwise-idle Pool engine (SWDGE path),
    off the SP sequencer (where a data-dependent store would block the
    prefetch loads queued behind it) and the shared HWDGE unit.
  - final row drains as 3 interleaved rank1+sqrt+store 512 slices, so the
    serial tail is one slice (ACT-issued store, empty queue behind it).

Scheme recap: core c is rotated so its own 1024 rows sit first; for each
512-row half h it computes columns [h*512, h*512+4608) of its rotated tile
(unit a covers column units a..a+8 mod 16; every pair is covered directly
or by the transpose of its mirror; the host mirrors the remaining 112
blocks). Output f16 (rel err ~5e-4 vs the 2e-2 gate), widened on the host.

Hardware pitfalls (this container's TRN2 + neuronxcc build):
  - InstTensorTensorReduce (fused DVE square+reduce) and ACT Square with
    accum_out both crash the device (NRT_EXEC_UNIT_UNRECOVERABLE); use
    plain Square + separate reduce_sum instead.
  - ACT Sqrt on negative inputs yields NaN (CoreSim asserts); clamp first.
"""

import sys

try:
    import concourse.bass as _probe  # noqa: F401
except ImportError:
    sys.path.insert(0, "/opt/trn_rl_repo")

import numpy as np

import concourse.bacc as bacc
import concourse.mybir as mybir
from concourse import tile
from concourse.bass_utils import run_bass_kernel_spmd

N = 8192          # number of points
D = 256           # feature dim
NCORES = 8
RPC = N // NCORES  # 1024 rows per core
U = 512            # unit = 512 rows/cols
SPAN_U = 9         # column units covered per 512-row half
NCOL = 10 * U      # columns of mt/nat each core holds (5120)
NG = 10            # sq groups of 512 rows each
NPAIR = 5          # nat load pairs (1024 rows)
NUNITS = N // U    # 16 global units
CW = 3 * U         # chunk width 1536

F16 = mybir.dt.float16
F32 = mybir.dt.float32

# entry schedule: (half, (units...)) — 1536-wide, ordered so entry i's
# chains/loads are prefetched during earlier entries
ENTRIES = [
    (0, (0, 1, 2)), (1, (1, 2, 3)),
    (0, (3, 4, 5)), (1, (4, 5, 6)),
    (0, (6, 7, 8)), (1, (7, 8, 9)),
]
# sq chains to emit inside each entry (groups 0-2 run before entry 0)
CHAINS = {0: (3, 4), 1: (5, 6), 2: (7,), 3: (8,), 4: (9,)}


def _build_nc(repeats=1, loop_n=None, stage_bufs=4, work_bufs=2):
    nc = bacc.Bacc(None, target_bir_lowering=False)
    mt_d = nc.dram_tensor("mt", [D, NCOL], F16, kind="ExternalInput")
    nat_d = nc.dram_tensor("nat", [NCOL, D], F16, kind="ExternalInput")
    eye_d = nc.dram_tensor("eye", [128, 128], F32, kind="ExternalInput")
    out_d = nc.dram_tensor("out", [RPC, NCOL], F16, kind="ExternalOutput")

    with tile.TileContext(nc) as tc:
        with (
            tc.tile_pool(name="big", bufs=1) as big,
            tc.tile_pool(name="work", bufs=work_bufs) as work,
            tc.tile_pool(name="stage", bufs=stage_bufs) as stage_pool,
            tc.tile_pool(name="ps", bufs=2, space="PSUM") as psum,
        ):
            if loop_n is not None:
                with tc.For_i(0, loop_n, 1):
                    _emit_body(nc, tc, big, work, stage_pool, psum,
                               mt_d, nat_d, eye_d, out_d)
            else:
                for _rep in range(repeats):
                    _emit_body(nc, tc, big, work, stage_pool, psum,
                               mt_d, nat_d, eye_d, out_d)

    nc.compile()
    return nc


def _emit_body(nc, tc, big, work, stage_pool, psum, mt_d, nat_d, eye_d, out_d):
    mt0 = big.tile([128, NCOL], F16, tag="mt0")
    mt1 = big.tile([128, NCOL], F16, tag="mt1")
    eye = big.tile([128, 128], F32, tag="eye")
    ones2 = big.tile([2, 128], F16, tag="ones2")
    # per-group sq tensors: a shared tile would create false WAR/RAW
    # couplings, serializing the pipeline
    sqp = []
    sqf = []
    for _g in range(NG):
        sqp_t = big.tile([128, 4], F32, tag=f"sqp{_g}")
        sqp.append(sqp_t)
        sqf_t = big.tile([2, U], F16, tag=f"sqf{_g}")
        sqf.append(sqf_t)
    half_own = big.tile([128, 8], F32, tag="half_own")

    natp = nat_d.rearrange("(q t p) d -> q p t d", q=NPAIR, p=128)
    gtp = {}
    for _q in range(NPAIR):
        gt_slot = big.tile([128, 8, 256], F16, tag=f"natp{_q}")
        gtp[_q] = gt_slot

    # initial loads: nat pairs 0-1 (sq groups 0..3) + mt units 0..3.
    # NOTE: reordering these (mt first) or splitting them finer lowers the
    # first-PE-op time from 7.2us to ~3.5us but LOSES overall: the idle is
    # then spread over several small mid-stream gaps, and every gap resets
    # the PE p-state ramp (3us of half-clock matmuls per gap). One
    # consolidated idle block at the start is the cheapest place to wait.
    nc.sync.dma_start(gtp[0][:], natp[0])
    nc.sync.dma_start(gtp[1][:], natp[1])
    nc.sync.dma_start(mt0[:, 0:2048], mt_d[0:128, 0:2048])
    nc.sync.dma_start(mt1[:, 0:2048], mt_d[128:256, 0:2048])
    nc.sync.dma_start(eye[:], eye_d[:])

    def emit_loads(ei):
        if ei == 0:
            nc.sync.dma_start(gtp[2][:], natp[2])
            nc.sync.dma_start(mt0[:, 2048:3072], mt_d[0:128, 2048:3072])
            nc.sync.dma_start(mt1[:, 2048:3072], mt_d[128:256, 2048:3072])
        elif ei == 1:
            nc.sync.dma_start(gtp[3][:], natp[3])
            nc.sync.dma_start(mt0[:, 3072:4096], mt_d[0:128, 3072:4096])
            nc.sync.dma_start(mt1[:, 3072:4096], mt_d[128:256, 3072:4096])
        elif ei == 2:
            nc.sync.dma_start(gtp[4][:], natp[4])
            nc.sync.dma_start(mt0[:, 4096:NCOL], mt_d[0:128, 4096:NCOL])
            nc.sync.dma_start(mt1[:, 4096:NCOL], mt_d[128:256, 4096:NCOL])

    def emit_sq_reduce(g):
        gt = gtp[g // 2][:, (g % 2) * 4:(g % 2) * 4 + 4, :]
        # square on ACT (plain Square — fused/accum variants crash this HW),
        # reduce on DVE
        msq = work.tile([128, 4, 256], F32, tag="msq")
        nc.scalar.activation(msq[:], gt,
                             mybir.ActivationFunctionType.Square)
        nc.vector.reduce_sum(sqp[g][:, 0:4].unsqueeze(2), msq[:],
                             axis=mybir.AxisListType.X)
        # -0.5*sq split hi/lo (exact to ~2^-22): hi source in cols 0:4,
        # f16 residual in cols 4:8 of one tile so a single PE transpose
        # flattens both rows
        mhl = work.tile([128, 8], F32, tag=f"mhl{g}")
        nc.vector.tensor_scalar_mul(mhl[:, 0:4], sqp[g][:, 0:4], -0.5)
        hi16 = work.tile([128, 4], F16, tag="hi16")
        nc.vector.tensor_copy(hi16[:], mhl[:, 0:4])
        hi32 = work.tile([128, 4], F32, tag="hi32")
        nc.vector.tensor_copy(hi32[:], hi16[:])
        nc.vector.tensor_sub(mhl[:, 4:8], mhl[:, 0:4], hi32[:])
        if g < 2:
            # own-rows 0.5*sq_i for the diagonal clamp
            nc.vector.tensor_scalar_mul(half_own[:, g * 4:(g + 1) * 4],
                                        sqp[g][:, 0:4], 0.5)
        return mhl

    def emit_sq_flatten(g, mhl):
        # one PE transpose [128, 8] -> [8, 128] (partition 4r+t holds row
        # r's tile-t slab), one f16 copy, one flatten DMA
        pt = psum.tile([8, 128], F32, tag="pst")
        nc.tensor.transpose(pt[:], mhl[:], eye[:])
        st = work.tile([8, 128], F16, tag="sqT")
        nc.vector.tensor_copy(st[:], pt[:])
        nc.sync.dma_start(
            sqf[g].rearrange("r (t i) -> r t i", t=4),
            st[:],
        )

    def emit_chain(g):
        emit_sq_flatten(g, emit_sq_reduce(g))

    def emit_kmms(ps, r, units):
        lhs0 = mt0[:, r * 128:(r + 1) * 128]
        lhs1 = mt1[:, r * 128:(r + 1) * 128]
        for s, u in enumerate(units):
            j = u * U
            nc.tensor.matmul(ps[:, s * U:(s + 1) * U], lhs0,
                             mt0[:, j:j + U], start=True, stop=False)
        for s, u in enumerate(units):
            j = u * U
            nc.tensor.matmul(ps[:, s * U:(s + 1) * U], lhs1,
                             mt1[:, j:j + U], start=False, stop=False)

    def emit_rank1(ps, units):
        for s, u in enumerate(units):
            nc.tensor.matmul(ps[:, s * U:(s + 1) * U], ones2[:],
                             sqf[u][:, :], start=False, stop=True)

    def emit_tail(ps, r, units):
        out_t = stage_pool.tile([128, CW], F16, tag="stage")
        bias = sqp[r // 4][:, r % 4:r % 4 + 1]
        if r // 4 in units:
            # diagonal block: clamp psum <= 0.5*sq_i so ACT Sqrt input
            # -2*psum + sq_i stays >= 0 under fp rounding
            s = units.index(r // 4)
            off = s * U + (r % 4) * 128
            dg = ps[:, off:off + 128]
            nc.vector.tensor_scalar_min(dg, dg, half_own[:, r:r + 1])
        nc.scalar.activation(
            out_t[:], ps[:],
            mybir.ActivationFunctionType.Sqrt,
            bias=bias, scale=-2.0,
        )
        # store from the Pool engine: SWDGE path, keeps the 24 output DMAs
        # off the SP sequencer and the shared HWDGE unit
        nc.gpsimd.dma_start(
            out_d[r * 128:(r + 1) * 128,
                  units[0] * U:units[0] * U + CW],
            out_t[:],
        )

    nc.vector.memset(ones2[:], 1.0)
    with tc.high_priority():
        emit_chain(0)
        emit_chain(1)

    for ei, (h, units) in enumerate(ENTRIES):
        rows = [4 * h + k for k in range(4)]
        chain = CHAINS.get(ei, ())
        if ei == 0:
            # runway: slot 0's k-matmuls stream while the group-2 chain
            # completes; rank-1s join once sqf lands
            ps0 = psum.tile([128, CW], F32, tag="ps")
            emit_kmms(ps0, rows[0], units)
            with tc.high_priority():
                emit_chain(2)
            emit_loads(0)
            ps1 = psum.tile([128, CW], F32, tag="ps")
            emit_kmms(ps1, rows[1], units)
            emit_rank1(ps0, units)
            emit_tail(ps0, rows[0], units)
            emit_rank1(ps1, units)
            emit_tail(ps1, rows[1], units)
            for idx, r in enumerate(rows[2:]):
                if idx < len(chain):
                    emit_chain(chain[idx])
                ps = psum.tile([128, CW], F32, tag="ps")
                emit_kmms(ps, r, units)
                emit_rank1(ps, units)
                emit_tail(ps, r, units)
            continue
        for idx, r in enumerate(rows):
            if idx == 0:
                emit_loads(ei)
            if idx in (1, 3) and len(chain) > idx // 2:
                emit_chain(chain[idx // 2])
            ps = psum.tile([128, CW], F32, tag="ps")
            emit_kmms(ps, r, units)
            if ei == len(ENTRIES) - 1 and idx == 3:
                # final row: interleave each 512 sub's rank-1 with its own
                # epilogue act + store so the drain tail is one slice, not
                # a whole 1536 chunk; the very last store is ACT-issued
                # (fast HWDGE gen, and the ACT queue behind it is empty)
                bias = sqp[r // 4][:, r % 4:r % 4 + 1]
                for s, u in enumerate(units):
                    nc.tensor.matmul(ps[:, s * U:(s + 1) * U], ones2[:],
                                     sqf[u][:, :], start=False, stop=True)
                    out_s = stage_pool.tile([128, U], F16, tag="stage_s")
                    nc.scalar.activation(
                        out_s[:], ps[:, s * U:(s + 1) * U],
                        mybir.ActivationFunctionType.Sqrt,
                        bias=bias, scale=-2.0,
                    )
                    eng = nc.scalar if s == len(units) - 1 else nc.gpsimd
                    eng.dma_start(
                        out_d[r * 128:(r + 1) * 128, u * U:(u + 1) * U],
                        out_s[:],
                    )
            else:
                emit_rank1(ps, units)
                emit_tail(ps, r, units)


_NC_CACHE = None


def _get_nc():
    global _NC_CACHE
    if _NC_CACHE is None:
        _NC_CACHE = _build_nc()
    return _NC_CACHE


def prep_inputs(mapping: np.ndarray) -> list:
    xh = mapping.astype(np.float16)
    eye = np.eye(128, dtype=np.float32)
    in_maps = []
    for c in range(NCORES):
        rot = np.roll(xh, -c * RPC, axis=0)
        natc = np.ascontiguousarray(rot[0:NCOL])
        mtc = np.ascontiguousarray(natc.T)
        in_maps.append({"mt": mtc, "nat": natc, "eye": eye})
    return in_maps


def kernel(mapping: np.ndarray, **_kwargs) -> np.ndarray:
    mapping = np.asarray(mapping, dtype=np.float32)
    assert mapping.shape == (N, D)
    in_maps = prep_inputs(mapping)

    nc = _get_nc()
    res = run_bass_kernel_spmd(nc, in_maps, core_ids=list(range(NCORES)))

    out = np.empty((N, N), dtype=np.float32)
    covered = np.zeros((NUNITS, NUNITS), dtype=bool)
    span = SPAN_U * U
    for c in range(NCORES):
        oc = res.results[c]["out"]  # [1024, 5120] f16
        for h in (0, 1):
            au = c * 2 + h
            block = oc[h * U:(h + 1) * U, h * U:h * U + span].astype(np.float32)
            gr0 = c * RPC + h * U
            gc0 = (c * RPC + h * U) % N
            first = min(span, N - gc0)
            out[gr0:gr0 + U, gc0:gc0 + first] = block[:, :first]
            if first < span:
                out[gr0:gr0 + U, 0:span - first] = block[:, first:]
            for cu in range(SPAN_U):
                covered[au, (au + cu) % NUNITS] = True
    for a in range(NUNITS):
        for b in range(NUNITS):
            if not covered[a, b]:
                out[a * U:(a + 1) * U, b * U:(b + 1) * U] = \
                    out[b * U:(b + 1) * U, a * U:(a + 1) * U].T
    return out


if __name__ == "__main__":
    rng = np.random.default_rng(0)
    x = rng.standard_normal((N, D)).astype(np.float32)
    o = kernel(mapping=x)
    sq = (x * x).sum(1)
    ref = np.sqrt(np.maximum(sq[:, None] + sq[None, :] - 2 * x @ x.T, 0))
    d = np.abs(o - ref)
    print("out", o.shape, o.dtype, "absmax diff", d.max(),
          "diag", np.abs(np.diag(o)).max())


# revision 21
# speedup vs baseline: 1.6343x; 1.5158x over previous
"""Pairwise Euclidean distance kernel for Trainium2 (8 NeuronCores, SPMD).

Problem: mapping [8192, 256] f32 -> out [8192, 8192] f32 where
out[i, j] = ||mapping[i] - mapping[j]||_2, via the GEMM identity
d2 = ||x_i||^2 + ||x_j||^2 - 2 <x_i, x_j>.

V3 = V2 (symmetric/triangle, f16 output) + sequencer/overhead engineering.
V2's trace showed PE.SEQ 100% busy (Matmult 42us exec + Ldweights 24us +
sems 12us), SP.SEQ 73us issuing 87 DMAs, HWDGE 54us of per-DMA fixed cost,
ACT 51us. V3:
  - [128, 1536] PSUM chunks (3 banks x 2 bufs + a separate 2-bank ring for
    the sq transposes): 24 uniform chunk-rows, no ragged 512 tails; 24
    epilogue ACT ops and 24 output DMAs instead of 40 each.
  - sq hi/lo flattened with ONE PE transpose per group ([128, 8] ->
    [8, 128], hi in cols 0:4, lo in 4:8) and ONE strided DMA per group.
  - input DMAs merged: mt in 1024/2048-col slices (8), nat in 1024-row
    pairs (5).
  - output stores issued from the otherwise-idle Pool engine (SWDGE path),
    off the SP sequencer (where a data-dependent store would block the
    prefetch loads queued behind it) and the shared HWDGE unit.
  - final row drains as 3 interleaved rank1+sqrt+store 512 slices, so the
    serial tail is one slice (ACT-issued store, empty queue behind it).

Scheme recap: core c is rotated so its own 1024 rows sit first; for each
512-row half h it computes columns [h*512, h*512+4608) of its rotated tile
(unit a covers column units a..a+8 mod 16; every pair is covered directly
or by the transpose of its mirror; the host mirrors the remaining 112
blocks). Output f16 (rel err ~5e-4 vs the 2e-2 gate), widened on the host.

Hardware pitfalls (this container's TRN2 + neuronxcc build):
  - InstTensorTensorReduce (fused DVE square+reduce) and ACT Square with
    accum_out both crash the device (NRT_EXEC_UNIT_UNRECOVERABLE); use
    plain Square + separate reduce_sum instead.
  - ACT Sqrt on negative inputs yields NaN (CoreSim asserts); clamp first.
"""

import sys

try:
    import concourse.bass as _probe  # noqa: F401
except ImportError:
    sys.path.insert(0, "/opt/trn_rl_repo")

import numpy as np

import concourse.bacc as bacc
import concourse.mybir as mybir
from concourse import tile
from concourse.bass_utils import run_bass_kernel_spmd

N = 8192          # number of points
D = 256           # feature dim
NCORES = 8
RPC = N // NCORES  # 1024 rows per core
U = 512            # unit = 512 rows/cols
SPAN_U = 9         # column units covered per 512-row half
NCOL = 10 * U      # columns of mt/nat each core holds (5120)
NG = 10            # sq groups of 512 rows each
NPAIR = 5          # nat load pairs (1024 rows)
NUNITS = N // U    # 16 global units
CW = 3 * U         # chunk width 1536

F16 = mybir.dt.float16
F32 = mybir.dt.float32

# entry schedule: (half, (units...)) — 1536-wide, ordered so entry i's
# chains/loads are prefetched during earlier entries
ENTRIES = [
    (0, (0, 1, 2)), (1, (1, 2, 3)),
    (0, (3, 4, 5)), (1, (4, 5, 6)),
    (0, (6, 7, 8)), (1, (7, 8, 9)),
]
# sq chains to emit inside each entry (groups 0-2 run before entry 0)
CHAINS = {0: (3, 4), 1: (5, 6), 2: (7,), 3: (8,), 4: (9,)}


def _build_nc(repeats=1, loop_n=None, stage_bufs=4, work_bufs=3):
    nc = bacc.Bacc(None, target_bir_lowering=False)
    mt_d = nc.dram_tensor("mt", [D, NCOL], F16, kind="ExternalInput")
    nat_d = nc.dram_tensor("nat", [NCOL, D], F16, kind="ExternalInput")
    eye_d = nc.dram_tensor("eye", [128, 128], F32, kind="ExternalInput")
    out_d = nc.dram_tensor("out", [RPC, NCOL], F16, kind="ExternalOutput")

    with tile.TileContext(nc) as tc:
        with (
            tc.tile_pool(name="big", bufs=1) as big,
            tc.tile_pool(name="work", bufs=work_bufs) as work,
            tc.tile_pool(name="stage", bufs=stage_bufs) as stage_pool,
            tc.tile_pool(name="ps", bufs=2, space="PSUM") as psum,
        ):
            if loop_n is not None:
                with tc.For_i(0, loop_n, 1):
                    _emit_body(nc, tc, big, work, stage_pool, psum,
                               mt_d, nat_d, eye_d, out_d)
            else:
                for _rep in range(repeats):
                    _emit_body(nc, tc, big, work, stage_pool, psum,
                               mt_d, nat_d, eye_d, out_d)

    nc.compile()
    return nc


def _emit_body(nc, tc, big, work, stage_pool, psum, mt_d, nat_d, eye_d, out_d):
    mt0 = big.tile([128, NCOL], F16, tag="mt0")
    mt1 = big.tile([128, NCOL], F16, tag="mt1")
    eye = big.tile([128, 128], F32, tag="eye")
    ones2 = big.tile([2, 128], F16, tag="ones2")
    # per-group sq tensors: a shared tile would create false WAR/RAW
    # couplings, serializing the pipeline
    sqp = []
    sqf = []
    for _g in range(NG):
        sqp_t = big.tile([128, 4], F32, tag=f"sqp{_g}")
        sqp.append(sqp_t)
        sqf_t = big.tile([2, U], F16, tag=f"sqf{_g}")
        sqf.append(sqf_t)
    # epilogue bias = sq_i + delta: the +4e-3 keeps -2*psum + bias > 0 even
    # in the diagonal blocks (fp round-off there is ~5.5e-4), so ACT Sqrt
    # never sees negatives and no DVE clamp is needed; costs sqrt(4e-3) =
    # 0.063 absolute on the (true-zero) diagonal, ~1e-4 relative elsewhere
    biasp = big.tile([128, 8], F32, tag="biasp")

    natp = nat_d.rearrange("(q t p) d -> q p t d", q=NPAIR, p=128)
    gtp = {}
    for _q in range(NPAIR):
        gt_slot = big.tile([128, 8, 256], F16, tag=f"natp{_q}")
        gtp[_q] = gt_slot

    # initial loads: nat pair 0 split in half so chain g0's square starts
    # after a 2KB/partition transfer (the start is chain-latency-bound),
    # then pair 1 + mt units 0..3.
    # NOTE: putting mt first lowers the first-PE-op time but LOSES overall:
    # the idle then spreads over several mid-stream gaps, and every gap
    # resets the PE p-state ramp (3us of half-clock matmuls per gap). One
    # consolidated idle block at the start is the cheapest place to wait.
    nc.sync.dma_start(gtp[0][:, 0:4, :], natp[0][:, 0:4, :])
    nc.sync.dma_start(gtp[0][:, 4:8, :], natp[0][:, 4:8, :])
    nc.sync.dma_start(gtp[1][:], natp[1])
    nc.sync.dma_start(mt0[:, 0:2048], mt_d[0:128, 0:2048])
    nc.sync.dma_start(mt1[:, 0:2048], mt_d[128:256, 0:2048])
    nc.sync.dma_start(eye[:], eye_d[:])

    def emit_loads(ei):
        # nat pairs arrive one entry before their groups' reduces so the
        # front-loaded squares never wait on data in the in-order ACT queue
        if ei == 0:
            nc.sync.dma_start(gtp[2][:], natp[2])
            nc.sync.dma_start(gtp[3][:], natp[3])
            nc.sync.dma_start(mt0[:, 2048:3072], mt_d[0:128, 2048:3072])
            nc.sync.dma_start(mt1[:, 2048:3072], mt_d[128:256, 2048:3072])
        elif ei == 1:
            nc.sync.dma_start(gtp[4][:], natp[4])
            nc.sync.dma_start(mt0[:, 3072:4096], mt_d[0:128, 3072:4096])
            nc.sync.dma_start(mt1[:, 3072:4096], mt_d[128:256, 3072:4096])
        elif ei == 2:
            nc.sync.dma_start(mt0[:, 4096:NCOL], mt_d[0:128, 4096:NCOL])
            nc.sync.dma_start(mt1[:, 4096:NCOL], mt_d[128:256, 4096:NCOL])

    def emit_sq_reduce(g):
        gt = gtp[g // 2][:, (g % 2) * 4:(g % 2) * 4 + 4, :]
        # square on ACT (plain Square — fused/accum variants crash this HW),
        # reduce on DVE
        msq = work.tile([128, 4, 256], F32, tag="msq")
        nc.scalar.activation(msq[:], gt,
                             mybir.ActivationFunctionType.Square)
        nc.vector.reduce_sum(sqp[g][:, 0:4].unsqueeze(2), msq[:],
                             axis=mybir.AxisListType.X)
        # -0.5*sq split hi/lo (exact to ~2^-22): hi source in cols 0:4,
        # f16 residual in cols 4:8 of one tile so a single PE transpose
        # flattens both rows
        mhl = work.tile([128, 8], F32, tag=f"mhl{g}")
        nc.vector.tensor_scalar_mul(mhl[:, 0:4], sqp[g][:, 0:4], -0.5)
        hi16 = work.tile([128, 4], F16, tag="hi16")
        nc.vector.tensor_copy(hi16[:], mhl[:, 0:4])
        hi32 = work.tile([128, 4], F32, tag="hi32")
        nc.vector.tensor_copy(hi32[:], hi16[:])
        nc.vector.tensor_sub(mhl[:, 4:8], mhl[:, 0:4], hi32[:])
        if g < 2:
            nc.vector.tensor_scalar_add(biasp[:, g * 4:(g + 1) * 4],
                                        sqp[g][:, 0:4], 4e-3)
        return mhl

    def emit_sq_flatten(g, mhl):
        # one PE transpose [128, 8] -> [8, 128] (partition 4r+t holds row
        # r's tile-t slab), one f16 copy, one flatten DMA
        pt = psum.tile([8, 128], F32, tag="pst")
        nc.tensor.transpose(pt[:], mhl[:], eye[:])
        st = work.tile([8, 128], F16, tag="sqT")
        nc.vector.tensor_copy(st[:], pt[:])
        nc.sync.dma_start(
            sqf[g].rearrange("r (t i) -> r t i", t=4),
            st[:],
        )

    def emit_chain(g):
        emit_sq_flatten(g, emit_sq_reduce(g))

    def emit_kmms(ps, r, units):
        lhs0 = mt0[:, r * 128:(r + 1) * 128]
        lhs1 = mt1[:, r * 128:(r + 1) * 128]
        for s, u in enumerate(units):
            j = u * U
            nc.tensor.matmul(ps[:, s * U:(s + 1) * U], lhs0,
                             mt0[:, j:j + U], start=True, stop=False)
        for s, u in enumerate(units):
            j = u * U
            nc.tensor.matmul(ps[:, s * U:(s + 1) * U], lhs1,
                             mt1[:, j:j + U], start=False, stop=False)

    def emit_rank1(ps, units):
        for s, u in enumerate(units):
            nc.tensor.matmul(ps[:, s * U:(s + 1) * U], ones2[:],
                             sqf[u][:, :], start=False, stop=True)

    def emit_tail(ps, r, units):
        out_t = stage_pool.tile([128, CW], F16, tag="stage")
        bias = biasp[:, r:r + 1]
        nc.scalar.activation(
            out_t[:], ps[:],
            mybir.ActivationFunctionType.Sqrt,
            bias=bias, scale=-2.0,
        )
        # store from the Pool engine: SWDGE path, keeps the 24 output DMAs
        # off the SP sequencer and the shared HWDGE unit
        nc.gpsimd.dma_start(
            out_d[r * 128:(r + 1) * 128,
                  units[0] * U:units[0] * U + CW],
            out_t[:],
        )

    nc.vector.memset(ones2[:], 1.0)
    with tc.high_priority():
        emit_chain(0)
        emit_chain(1)

    mhls = {}
    for ei, (h, units) in enumerate(ENTRIES):
        rows = [4 * h + k for k in range(4)]
        if ei == 0:
            # runway: slot 0's k-matmuls stream while the group-2 chain
            # completes; rank-1s join once sqf lands. Reduces for groups
            # 3-6 are interleaved between tails (their squares slot into
            # the ACT queue between epilogues); each group's PE flatten
            # transpose trails its reduce by ~2 rows so the in-order PE
            # never parks on the chain.
            ps0 = psum.tile([128, CW], F32, tag="ps")
            emit_kmms(ps0, rows[0], units)
            with tc.high_priority():
                emit_chain(2)
            emit_loads(0)
            ps1 = psum.tile([128, CW], F32, tag="ps")
            emit_kmms(ps1, rows[1], units)
            emit_rank1(ps0, units)
            emit_tail(ps0, rows[0], units)
            mhls[3] = emit_sq_reduce(3)
            emit_rank1(ps1, units)
            emit_tail(ps1, rows[1], units)
            mhls[4] = emit_sq_reduce(4)
            mhls[5] = emit_sq_reduce(5)
            for idx, r in enumerate(rows[2:]):
                emit_sq_flatten(3 + idx, mhls[3 + idx])
                if idx == 1:
                    mhls[6] = emit_sq_reduce(6)
                ps = psum.tile([128, CW], F32, tag="ps")
                emit_kmms(ps, r, units)
                emit_rank1(ps, units)
                emit_tail(ps, r, units)
            continue
        for idx, r in enumerate(rows):
            if idx == 0:
                emit_loads(ei)
            if ei == 1:
                # flatten g5..g8 at idx 0..3; reduce g7..g9 at idx 0..2
                emit_sq_flatten(5 + idx, mhls[5 + idx])
                if idx < 3:
                    mhls[7 + idx] = emit_sq_reduce(7 + idx)
            elif ei == 2 and idx == 0:
                emit_sq_flatten(9, mhls[9])
            ps = psum.tile([128, CW], F32, tag="ps")
            emit_kmms(ps, r, units)
            if ei == len(ENTRIES) - 1 and idx == 3:
                # final row: interleave each 512 sub's rank-1 with its own
                # epilogue act + store so the drain tail is one slice, not
                # a whole 1536 chunk; the very last store is ACT-issued
                # (fast HWDGE gen, and the ACT queue behind it is empty)
                bias = biasp[:, r:r + 1]
                for s, u in enumerate(units):
                    nc.tensor.matmul(ps[:, s * U:(s + 1) * U], ones2[:],
                                     sqf[u][:, :], start=False, stop=True)
                    out_s = stage_pool.tile([128, U], F16, tag="stage_s")
                    nc.scalar.activation(
                        out_s[:], ps[:, s * U:(s + 1) * U],
                        mybir.ActivationFunctionType.Sqrt,
                        bias=bias, scale=-2.0,
                    )
                    eng = nc.scalar if s == len(units) - 1 else nc.gpsimd
                    eng.dma_start(
                        out_d[r * 128:(r + 1) * 128, u * U:(u + 1) * U],
                        out_s[:],
                    )
            else:
                emit_rank1(ps, units)
                emit_tail(ps, r, units)


_NC_CACHE = None


def _get_nc():
    global _NC_CACHE
    if _NC_CACHE is None:
        _NC_CACHE = _build_nc()
    return _NC_CACHE


def prep_inputs(mapping: np.ndarray) -> list:
    xh = mapping.astype(np.float16)
    eye = np.eye(128, dtype=np.float32)
    in_maps = []
    for c in range(NCORES):
        rot = np.roll(xh, -c * RPC, axis=0)
        natc = np.ascontiguousarray(rot[0:NCOL])
        mtc = np.ascontiguousarray(natc.T)
        in_maps.append({"mt": mtc, "nat": natc, "eye": eye})
    return in_maps


def kernel(mapping: np.ndarray, **_kwargs) -> np.ndarray:
    mapping = np.asarray(mapping, dtype=np.float32)
    assert mapping.shape == (N, D)
    in_maps = prep_inputs(mapping)

    nc = _get_nc()
    res = run_bass_kernel_spmd(nc, in_maps, core_ids=list(range(NCORES)))

    out = np.empty((N, N), dtype=np.float32)
    covered = np.zeros((NUNITS, NUNITS), dtype=bool)
    span = SPAN_U * U
    for c in range(NCORES):
        oc = res.results[c]["out"]  # [1024, 5120] f16
        for h in (0, 1):
            au = c * 2 + h
            block = oc[h * U:(h + 1) * U, h * U:h * U + span].astype(np.float32)
            gr0 = c * RPC + h * U
            gc0 = (c * RPC + h * U) % N
            first = min(span, N - gc0)
            out[gr0:gr0 + U, gc0:gc0 + first] = block[:, :first]
            if first < span:
                out[gr0:gr0 + U, 0:span - first] = block[:, first:]
            for cu in range(SPAN_U):
                covered[au, (au + cu) % NUNITS] = True
    for a in range(NUNITS):
        for b in range(NUNITS):
            if not covered[a, b]:
                out[a * U:(a + 1) * U, b * U:(b + 1) * U] = \
                    out[b * U:(b + 1) * U, a * U:(a + 1) * U].T
    return out


if __name__ == "__main__":
    rng = np.random.default_rng(0)
    x = rng.standard_normal((N, D)).astype(np.float32)
    o = kernel(mapping=x)
    sq = (x * x).sum(1)
    ref = np.sqrt(np.maximum(sq[:, None] + sq[None, :] - 2 * x @ x.T, 0))
    d = np.abs(o - ref)
    print("out", o.shape, o.dtype, "absmax diff", d.max(),
          "diag", np.abs(np.diag(o)).max())
